# revision 31
# baseline (speedup 1.0000x reference)
"""Distributed GQA attention prefill for TRN2 (8 NeuronCores).

Problem: T=2048, D=4096, N=32 query heads, K=8 kv heads, H=128.
    q = x @ w_q; k = x @ w_k; v = x @ w_v   (fused in the reference)
    rope(q), rope(k); causal GQA attention; out = o @ w_o

Sharding: tensor-parallel over heads for QKV+attention (core c owns
query heads 4c..4c+3 and kv head c; w_q sharded on N, w_k/w_v on K,
x replicated) — but the o_proj is sharded over OUTPUT COLUMNS: the
per-panel attention outputs oT (2MB per core in total) are
AllGathered, and each core contracts all 32 heads against its
[4096, 512] w_o column shard. The head-sum reduction thus happens
inside the o_proj matmul in fp32 PSUM — there is no ReduceScatter of
16MB o_proj partials and no reduce arithmetic on the collective
cores, which profiling showed to be the collective bottleneck. The
host concatenates the 8 per-core [T, 512] column shards.

Single merged pipeline. QKV runs in four 512-wide panels (full-rate
matmuls); attention runs in five panels (512,512,512,256,256) so the
trailing AllGather+o_proj tail stays small. Causality means a
panel's attention needs only k/v panels <= it, so the collectives
stream from ~100us and drain concurrently with compute.

Scheduling details that matter (each fixed a measured stall):
- S(g) interleaves with the q(g+1) projection so the scalar-engine
  exp chain hides under projection matmuls; PV lags S by one head to
  bound live P tiles.
- Diagonal S blocks use reduced moving width (within-panel causal
  skip) and are emitted first so the causal-mask multiply never
  stalls the vector queue.
- Each panel's o_proj is held back and emitted inside the NEXT
  panel's S phase (its AllGather completes meanwhile), so its
  PSUM->SBUF copies never sit ahead of the next panel's rope/copy
  work in the vector FIFO.
- The o_proj and the v transpose borrow the QKV PSUM pool (4 banks,
  strict FIFO rotation is deadlock-checked in emission order).
- RoPE halfswaps ride the sync queue (gpsimd carries the collectives
  and is strictly in-order).
- AllGather chunk loads ride the gpsimd queue only: they must wait
  for the collective anyway, and on sync/scalar the scheduler hoists
  them ahead of rope swaps / exps, blocking those queues ~30us per
  panel. (Tail-panel loads split with scalar, whose exps are done.)
Measured ~522-537us HW exec on trn2 (8 cores), rel err 6.6e-3
(baseline two-phase ReduceScatter version: ~567-580us).
"""

import numpy as np
import ml_dtypes

T, D, NH, KH, H = 2048, 4096, 32, 8, 128
THETA = 10000.0
G = NH // KH          # 4 query heads per core
N_CORES = 8
NQP = 4               # QKV panels (512 wide)
NTB = T // 128        # 16 t/s blocks
NDB = D // 128        # 32 d blocks
NHB = NH * H // 128   # 32 hn blocks (all heads)
DSH = D // N_CORES    # 512 o_proj output columns per core
SCALE = 1.0 / float(np.sqrt(H))
VEXT_STRIDE = 132     # v_ext row stride (129 used, padded)
# Attention panels as (first t-block, n t-blocks, QKV-range or None).
# A QKV range (qb0, qnb) projects t-blocks qb0..qb0+qnb with full-width
# matmuls; panel 3 projects blocks 12-15 and panel 4 reuses them.
ATT_PANELS = [(0, 4, (0, 4)), (4, 4, (4, 4)), (8, 4, (8, 4)),
              (12, 2, (12, 4)), (14, 2, None)]

_NC_CACHE = {}


def _build_nc():
    import concourse.mybir as mybir
    import concourse.tile as tile
    from concourse import bacc
    from concourse.masks import make_identity

    BF16 = mybir.dt.bfloat16
    F32 = mybir.dt.float32
    EXP = mybir.ActivationFunctionType.Exp

    nc = bacc.Bacc("TRN2", target_bir_lowering=False, debug=False,
                   num_devices=N_CORES)

    xt_ext = nc.dram_tensor("xt", [128, NQP, NDB, 512], BF16,
                            kind="ExternalInput")
    wq_ext = nc.dram_tensor("wq", [128, NDB, G * H], BF16,
                            kind="ExternalInput")
    wk_ext = nc.dram_tensor("wk", [128, NDB, H], BF16, kind="ExternalInput")
    wv_ext = nc.dram_tensor("wv", [128, NDB, H], BF16, kind="ExternalInput")
    wo_ext = nc.dram_tensor("wo", [128, NHB, DSH], BF16,
                            kind="ExternalInput")
    cos_ext = nc.dram_tensor("cos_t", [H, T], BF16, kind="ExternalInput")
    sin_ext = nc.dram_tensor("sin_t", [H, T], BF16, kind="ExternalInput")
    mask_ext = nc.dram_tensor("maskp", [128, 128], BF16, kind="ExternalInput")
    out_ext = nc.dram_tensor("out", [T, DSH], BF16, kind="ExternalOutput")

    with tile.TileContext(nc) as tc:
        with (
            tc.tile_pool(name="consts", bufs=1) as consts,
            tc.tile_pool(name="persist", bufs=1) as persist,
            tc.tile_pool(name="dram", bufs=1, space="DRAM") as dram,
            tc.tile_pool(name="qtp", bufs=8) as qtp,
            tc.tile_pool(name="rawp", bufs=2) as rawp,
            tc.tile_pool(name="ropep", bufs=4) as ropep,
            tc.tile_pool(name="ptp", bufs=38) as ptp,
            tc.tile_pool(name="otp", bufs=8) as otp,
            tc.tile_pool(name="agp", bufs=8) as agp,
            tc.tile_pool(name="outp", bufs=3) as outp,
            tc.tile_pool(name="scp", bufs=6) as scp,
            tc.tile_pool(name="bigps", bufs=4, space="PSUM") as big_ps,
            tc.tile_pool(name="sps", bufs=2, space="PSUM") as sps,
            tc.tile_pool(name="smallps", bufs=2, space="PSUM") as smallps,
        ):
            cos_sb = consts.tile([H, T], BF16)
            sin_sb = consts.tile([H, T], BF16)
            mask_sb = consts.tile([128, 128], BF16)
            ident = consts.tile([128, 128], BF16)
            make_identity(nc, ident[:])

            kT = persist.tile([128, T], BF16)
            v_ext = persist.tile([128, NTB, VEXT_STRIDE], BF16)
            xbuf = persist.tile([128, NDB, 512], BF16)
            wq_sb = persist.tile([128, NDB, G * H], BF16)
            wk_sb = persist.tile([128, NDB, H], BF16)
            wv_sb = persist.tile([128, NDB, H], BF16)
            wo_sb = persist.tile([128, NHB, DSH], BF16)

            ag_in = [dram.tile([G * H, nblk * 128], BF16, tag=f"agi{p}",
                               name=f"agi{p}")
                     for p, (tb0, nblk, q) in enumerate(ATT_PANELS)]
            ag_out = [dram.tile([NH * H, nblk * 128], BF16, tag=f"ago{p}",
                                name=f"ago{p}", addr_space="Shared")
                      for p, (tb0, nblk, q) in enumerate(ATT_PANELS)]

            # ---- startup DMAs ----
            # scalar queue: weights, finely chunked so the first QKV
            # matmuls can start as soon as wk d-block 0 lands
            for cch in range(4):
                nc.scalar.dma_start(
                    out=wk_sb[:, 8 * cch:8 * (cch + 1), :],
                    in_=wk_ext[:, 8 * cch:8 * (cch + 1), :])
            nc.scalar.dma_start(out=wv_sb[:], in_=wv_ext[:])
            for cch in range(4):
                nc.scalar.dma_start(
                    out=wq_sb[:, 8 * cch:8 * (cch + 1), :],
                    in_=wq_ext[:, 8 * cch:8 * (cch + 1), :])
            nc.scalar.dma_start(out=wo_sb[:], in_=wo_ext[:])
            # gpsimd queue: rope tables + mask (small, needed early;
            # all later gpsimd work is collectives)
            nc.gpsimd.dma_start(out=cos_sb[:], in_=cos_ext[:])
            nc.gpsimd.dma_start(out=sin_sb[:], in_=sin_ext[:])
            nc.gpsimd.dma_start(out=mask_sb[:], in_=mask_ext[:])
            # sync queue: x panel 0; first chunks small to start compute
            for c0, c1 in ((0, 4), (4, 8), (8, 16), (16, 24), (24, 32)):
                nc.sync.dma_start(
                    out=xbuf[:, c0:c1, :],
                    in_=xt_ext[:, 0, c0:c1, :])
            nc.vector.memset(v_ext[:, :, 128:129], 1.0)

            def rope(tl, dsl, tsl, pw):
                """tl[:, dsl] = tl[:, dsl]*cos[tsl] + halfswap(.)*sin[tsl]."""
                sw = ropep.tile([128, 512], BF16, tag="ropesw")
                t1 = ropep.tile([128, 512], BF16, tag="ropet1")
                # sync queue, NOT gpsimd: the gpsimd queue is in-order and
                # carries the collectives — a halfswap queued behind a
                # collective would stall the next panel's attention.
                nc.sync.dma_start(out=sw[0:64, 0:pw], in_=tl[64:128, dsl])
                nc.sync.dma_start(out=sw[64:128, 0:pw], in_=tl[0:64, dsl])
                nc.vector.tensor_tensor(
                    out=t1[:, 0:pw], in0=tl[:, dsl], in1=cos_sb[:, tsl],
                    op=mybir.AluOpType.mult)
                nc.vector.tensor_tensor(
                    out=sw[:, 0:pw], in0=sw[:, 0:pw], in1=sin_sb[:, tsl],
                    op=mybir.AluOpType.mult)
                nc.vector.tensor_tensor(
                    out=tl[:, dsl], in0=t1[:, 0:pw], in1=sw[:, 0:pw],
                    op=mybir.AluOpType.add)

            def qkv_group(w_tile, csl, dst):
                """dst = w_tile[:, :, csl].T @ x_panel (32 d-block accum)."""
                ps = big_ps.tile([128, 512], F32, tag="big")
                for db in range(NDB):
                    nc.tensor.matmul(
                        ps[:], w_tile[:, db, csl], xbuf[:, db, :],
                        start=(db == 0), stop=(db == NDB - 1))
                nc.vector.tensor_copy(dst, ps[:])

            def s_block(qTg, co, tb0, pw, sb, pts_g):
                """S^T block + exp; diagonal blocks use reduced width."""
                jj = sb - tb0
                c0 = jj * 128 if jj > 0 else 0
                w = pw - c0
                ps_s = sps.tile([128, 512], F32, tag="s")
                nc.tensor.matmul(
                    ps_s[:, 0:w], kT[:, sb * 128:(sb + 1) * 128],
                    qTg[:, co + c0:co + pw], start=True, stop=True)
                pt = ptp.tile([128, 512], BF16, tag="pt")
                nc.scalar.activation(pt[:, c0:pw], ps_s[:, 0:w], EXP,
                                     scale=SCALE)
                if jj >= 0:
                    nc.vector.tensor_tensor(
                        out=pt[:, jj * 128:(jj + 1) * 128],
                        in0=pt[:, jj * 128:(jj + 1) * 128],
                        in1=mask_sb[:],
                        op=mybir.AluOpType.mult)
                pts_g[sb] = pt

            def pv_head(g, tb0, nblk, pts_g, oT_t):
                """PV + normalize + transpose for head g's t-blocks."""
                for j in range(nblk):
                    tb = tb0 + j
                    ps_pv = smallps.tile([128, 129], F32, tag="sm")
                    for sb in range(tb + 1):
                        nc.tensor.matmul(
                            ps_pv[:],
                            pts_g[sb][:, j * 128:(j + 1) * 128],
                            v_ext[:, sb, 0:129],
                            start=(sb == 0), stop=(sb == tb),
                            skip_group_check=True)
                    rc = scp.tile([128, 1], F32, tag="rc")
                    nc.vector.reciprocal(rc[:], ps_pv[:, 128:129])
                    ob = scp.tile([128, 128], BF16, tag="ob")
                    nc.vector.tensor_scalar_mul(ob[:], ps_pv[:, 0:128], rc[:])
                    ps_tr = smallps.tile([128, 128], BF16, tag="sm")
                    nc.tensor.transpose(ps_tr[:], ob[:], ident[:])
                    nc.vector.tensor_copy(
                        oT_t[g][:, j * 128:(j + 1) * 128], ps_tr[:])

            def ag_launch(p, nblk, oT_t):
                """Stage this panel's oT and trigger its AllGather.

                Stores ride the gpsimd queue: they only need to finish
                before THIS panel's collective, which the in-order queue
                guarantees, and they stay clear of the compute queues.
                """
                pw = nblk * 128
                for g in range(G):
                    nc.gpsimd.dma_start(
                        out=ag_in[p][g * H:(g + 1) * H, 0:pw],
                        in_=oT_t[g][:, 0:pw])
                nc.gpsimd.collective_compute(
                    "AllGather",
                    mybir.AluOpType.bypass,
                    replica_groups=[list(range(N_CORES))],
                    ins=[ag_in[p].opt()],
                    outs=[ag_out[p].opt()],
                )

            def oproj(p, tb0, nblk):
                """Column-sharded o_proj for panel p's t-blocks.

                Contracts all 32 heads of the AllGathered oT against the
                [4096, DSH] w_o column shard: out[t, :] = sum_hn
                oT_all[hn, t] * wo[hn, :]. hn-outer streaming; nblk PSUM
                accumulators (from the shared big pool) live at once.
                """
                pw = nblk * 128
                ods = [big_ps.tile([128, DSH], F32, tag="big",
                                   name=f"od{p}_{jj}")
                       for jj in range(nblk)]
                # chunk loads ride the gpsimd queue: they must wait for
                # this panel's AllGather anyway, and on any other queue
                # the scheduler can hoist them ahead of rope swaps / exps,
                # whose queue then blocks ~30us on the collective. Tail
                # panels (1 matmul per chunk) split with scalar — all of
                # that panel's exps are already emitted — to halve the
                # dispatch-bound chunk rate.
                for hnb in range(NHB):
                    agt = agp.tile([128, 512], BF16, tag="ag")
                    q = nc.scalar if (nblk <= 2 and hnb % 2) else nc.gpsimd
                    q.dma_start(out=agt[:, 0:pw],
                                in_=ag_out[p][hnb * 128:(hnb + 1) * 128, :])
                    for j in range(nblk):
                        nc.tensor.matmul(
                            ods[j][:],
                            agt[:, j * 128:(j + 1) * 128],
                            wo_sb[:, hnb, :],
                            start=(hnb == 0), stop=(hnb == NHB - 1),
                            skip_group_check=True)
                for j in range(nblk):
                    osb = outp.tile([128, DSH], BF16, tag="osb")
                    nc.vector.tensor_copy(osb[:], ods[j][:])
                    nc.scalar.dma_start(
                        out=out_ext[(tb0 + j) * 128:(tb0 + j + 1) * 128, :],
                        in_=osb[:])

            # ================= merged per-panel pipeline =================
            pending = None    # (p, tb0, nblk) o_proj held from prev panel
            qT = [None] * G   # live q tiles (shared by panels 3 and 4)
            vraw = None
            for ap, (tb0, nblk, qkv) in enumerate(ATT_PANELS):
                pw = nblk * 128
                t0 = tb0 * 128
                co = t0 % 512 if qkv is None else 0
                tsl = slice(t0, t0 + pw)
                n_sb = tb0 + nblk
                if qkv is not None:
                    qb0, qnb = qkv
                    qpw = qnb * 128
                    qtsl = slice(qb0 * 128, qb0 * 128 + qpw)
                    qkv_group(wk_sb, slice(0, H), kT[:, qtsl])
                    rope(kT, qtsl, qtsl, qpw)
                    vraw = rawp.tile([128, 512], BF16, tag="raw")
                    qkv_group(wv_sb, slice(0, H), vraw[:, 0:qpw])

                oT_t = [otp.tile([128, 512], BF16, tag="oT",
                                 name=f"oT{ap}_{gg}")
                        for gg in range(G)]
                pts = [dict() for _ in range(G)]
                # diagonal s-blocks first so the mask mult (vector queue)
                # never waits on a long exp chain
                sb_order = list(range(tb0, n_sb)) + list(range(tb0))
                for g in range(G):
                    if qkv is not None:
                        qTg = qtp.tile([128, 512], BF16, tag="qT",
                                       name=f"qT{ap}_{g}")
                        qkv_group(wq_sb, slice(g * H, (g + 1) * H),
                                  qTg[:, 0:qpw])
                        rope(qTg, slice(0, qpw), qtsl, qpw)
                        qT[g] = qTg
                        if g == G - 1 and ap + 1 < NQP:
                            # first chunk of the next x panel early so the
                            # next panel's k projection starts on time
                            # (WAR on this panel's q reads is tracked)
                            nc.sync.dma_start(
                                out=xbuf[:, 0:8, :],
                                in_=xt_ext[:, ap + 1, 0:8, :])
                    for sb in sb_order:
                        s_block(qT[g], co, tb0, pw, sb, pts[g])
                    if g == 1:
                        # v transpose (feeds PV diag blocks); borrows the
                        # big PSUM pool (idle slots at this point)
                        for jb in range(nblk):
                            pst = big_ps.tile([128, 128], BF16, tag="big",
                                              name=f"vtr{ap}_{jb}")
                            nc.tensor.transpose(
                                pst[:],
                                vraw[:, co + jb * 128:co + (jb + 1) * 128],
                                ident[:])
                            nc.vector.tensor_copy(
                                v_ext[:, tb0 + jb, 0:128], pst[:])
                    if g == G - 1 and pending is not None:
                        # held-over o_proj: its AllGather has been running
                        # since the previous panel's end
                        oproj(*pending)
                        pending = None
                    if g >= 1:
                        pv_head(g - 1, tb0, nblk, pts[g - 1], oT_t)
                        pts[g - 1] = None
                if qkv is not None and ap + 1 < NQP:
                    # rest of the next x panel: after the o_proj hook so
                    # it never delays the AllGather chunk loads
                    for dc in range(1, 4):
                        nc.sync.dma_start(
                            out=xbuf[:, 8 * dc:8 * (dc + 1), :],
                            in_=xt_ext[:, ap + 1, 8 * dc:8 * (dc + 1), :])
                pv_head(G - 1, tb0, nblk, pts[G - 1], oT_t)
                ag_launch(ap, nblk, oT_t)
                pending = (ap, tb0, nblk)
                if ap == len(ATT_PANELS) - 1:
                    oproj(*pending)

    nc.compile()
    return nc


def get_nc():
    if "nc" not in _NC_CACHE:
        _NC_CACHE["nc"] = _build_nc()
    return _NC_CACHE["nc"]


def make_in_maps(x, positions, w_q, w_k, w_v, w_o):
    """Host-side sharding + RoPE table / mask precompute."""
    x = np.ascontiguousarray(np.asarray(x, np.float32))
    positions = np.asarray(positions)

    half = H // 2
    inv_freq = 1.0 / (THETA ** (np.arange(half, dtype=np.float32) / half))
    ang = positions.astype(np.float32)[:, None] * inv_freq[None, :]  # [T, 64]
    cos = np.cos(ang)   # [T, 64]
    sin = np.sin(ang)
    cos_t = np.empty((H, T), np.float32)
    sin_t = np.empty((H, T), np.float32)
    cos_t[0:half] = cos.T
    cos_t[half:] = cos.T
    sin_t[0:half] = -sin.T
    sin_t[half:] = sin.T
    cos_t = cos_t.astype(ml_dtypes.bfloat16)
    sin_t = sin_t.astype(ml_dtypes.bfloat16)

    # mask[s, t] = 1 if s <= t (lower-left of P^T allowed region)
    idx = np.arange(128)
    maskp = (idx[:, None] <= idx[None, :]).astype(ml_dtypes.bfloat16)

    xt = x.astype(ml_dtypes.bfloat16).T  # [D, T]
    xt4 = np.ascontiguousarray(
        xt.reshape(NDB, 128, NQP, 512).transpose(1, 2, 0, 3))
    w_q = np.asarray(w_q, np.float32).reshape(D, NH, H).astype(
        ml_dtypes.bfloat16)
    w_k = np.asarray(w_k, np.float32).reshape(D, KH, H).astype(
        ml_dtypes.bfloat16)
    w_v = np.asarray(w_v, np.float32).reshape(D, KH, H).astype(
        ml_dtypes.bfloat16)
    # full [N*H, D] with row hn = n*128+h — matches the AllGather row
    # order (core c contributes heads 4c..4c+3)
    w_o_full = np.asarray(w_o, np.float32).reshape(NH * H, D).astype(
        ml_dtypes.bfloat16)

    def blk(w):
        """[D, n] -> [128, NDB, n] with row d = a*128 + p."""
        return np.ascontiguousarray(
            w.reshape(NDB, 128, -1).transpose(1, 0, 2))

    in_maps = []
    for c in range(N_CORES):
        in_maps.append({
            "xt": xt4,
            "wq": blk(w_q[:, G * c:G * (c + 1), :].reshape(D, G * H)),
            "wk": blk(w_k[:, c, :]),
            "wv": blk(w_v[:, c, :]),
            "wo": np.ascontiguousarray(
                w_o_full[:, DSH * c:DSH * (c + 1)]
                .reshape(NHB, 128, DSH).transpose(1, 0, 2)),
            "cos_t": cos_t,
            "sin_t": sin_t,
            "maskp": maskp,
        })
    return in_maps


def assemble_output(results):
    """results: list of 8 per-core dicts with 'out' [T, D/8] bf16."""
    out = np.empty((T, D), np.float32)
    for c in range(N_CORES):
        out[:, DSH * c:DSH * (c + 1)] = np.asarray(
            results[c]["out"], np.float32)
    return out


def kernel(x, positions, w_q, w_k, w_v, w_o):
    from concourse.bass_utils import run_bass_kernel_spmd

    nc = get_nc()
    in_maps = make_in_maps(x, positions, w_q, w_k, w_v, w_o)
    res = run_bass_kernel_spmd(nc, in_maps, core_ids=list(range(N_CORES)))
    return assemble_output(res.results)
